# revision 18
# baseline (speedup 1.0000x reference)
"""Trainium2 Bass kernel: 16-head MHA (B=4, S=2048, E=1024, Dh=64), 8 cores.

Sharding: core c handles batch b = c//2 and head-group g = c%2 (8 heads).
Each core computes its 8 heads' attention plus the partial output
projection in transposed layout oT[e, s]; the host sums the two
head-group partials per batch, transposes, and adds bo.

Per-core dataflow (all matmuls bf16 with fp32 PSUM accumulation):
  qT/kT[d, s]  = Wq/Wk.T @ xT          (per head-pair, d stacked 2x64)
  v[t, hd]     = xT.T @ Wv + ones.T@bv (natural layout, + ones col for colsum)
  scoresT[t,s] = kT.T @ qT   (row-tiled: 2 heads in rows 0-63 / 64-127)
  expT         = exp(0.125 * scoresT)  (ScalarE, cast to bf16)
  zT_un[d,s],colsum[s] = v_aug.T @ expT  (M=65: row 64 = colsum)
  zT           = zT_un * bcast(1/colsum) (bcast via K=1 matmul)
  oT[e, s]    += Wo_h.T @ zT_h  (accumulated over the core's 8 heads)
"""

import numpy as np
import ml_dtypes

B, S, E = 4, 2048, 1024
H, Dh = 16, 64
N_CORES = 8
HPC = 8          # heads per core
MP = 4           # head-pairs per core
SC, SCW = 4, 512  # s-chunks
TC, TCW = 16, 128  # t-chunks
KE = 8           # k-tiles over E
ECN = 8          # e-chunks of 128 (outT partition tiles)

BF16 = ml_dtypes.bfloat16

_PROG = None


def _build_program(repeats=None, timing=False, parts=3):
    """Emit the Bass/Tile program. Returns (nc, names_dict).

    repeats: if set, wrap the whole body in a For_i loop (for marginal
    per-iteration HW timing; not used by the graded kernel() path).
    timing: demote the real output to internal DRAM and expose a tiny
    dummy output instead, so timing calls don't pay output transfers.
    parts: 1 = projections only, 2 = + attention, 3 = full (default).
    """
    from contextlib import ExitStack

    import concourse.mybir as mybir
    import concourse.tile as tile
    from concourse import bacc

    dt = mybir.dt
    AF = mybir.ActivationFunctionType
    OP = mybir.AluOpType

    nc = bacc.Bacc(None, target_bir_lowering=False, debug=False)
    with tile.TileContext(nc) as tc:
        with tc.tile_pool(name="dram", bufs=1, space="DRAM") as dram:
            xT_d = dram.tile([E, S], dt.bfloat16, kind="ExternalInput")
            wq_d = dram.tile([E, HPC * Dh], dt.bfloat16, kind="ExternalInput")
            wk_d = dram.tile([E, HPC * Dh], dt.bfloat16, kind="ExternalInput")
            wv_d = dram.tile([E, HPC * Dh], dt.bfloat16, kind="ExternalInput")
            wo_d = dram.tile([Dh, HPC, E], dt.bfloat16, kind="ExternalInput")
            bq_d = dram.tile([128, MP], dt.float32, kind="ExternalInput")
            bk_d = dram.tile([128, MP], dt.float32, kind="ExternalInput")
            bv_d = dram.tile([1, HPC * Dh], dt.bfloat16, kind="ExternalInput")
            if timing:
                oT_d = dram.tile([E, S], dt.float32, kind="Internal")
                dummy_d = dram.tile([1, 4], dt.bfloat16, kind="ExternalOutput")
            else:
                oT_d = dram.tile([E, S], dt.float32, kind="ExternalOutput")
                dummy_d = None

            with (
                tc.tile_pool(name="const", bufs=1) as const,
                tc.tile_pool(name="work", bufs=2) as work,
                tc.tile_pool(name="norm1", bufs=1) as norm1,
                tc.tile_pool(name="zpool", bufs=1) as zpool,
                tc.tile_pool(name="psum_sT", bufs=2, space="PSUM") as psum_sT,
                tc.tile_pool(name="psum_z", bufs=4, space="PSUM") as psum_z,
                ExitStack() as _es,
            ):
                if repeats is not None:
                    _es.enter_context(tc.For_i(
                        0, repeats, 1,
                        hint_engines=(
                            mybir.EngineType.PE, mybir.EngineType.Activation,
                            mybir.EngineType.DVE, mybir.EngineType.SP,
                            mybir.EngineType.Pool,
                        ),
                    ))
                # ---- persistent SBUF ----
                xT = const.tile([128, KE, S], dt.bfloat16)
                wq = const.tile([128, KE, HPC * Dh], dt.bfloat16)
                wk = const.tile([128, KE, HPC * Dh], dt.bfloat16)
                wv = const.tile([128, KE, HPC * Dh], dt.bfloat16)
                wo = const.tile([Dh, HPC, E], dt.bfloat16)
                bqk = const.tile([128, 2 * MP], dt.float32)
                onesbv = const.tile([1, HPC * Dh + 128], dt.bfloat16)
                qT2 = const.tile([128, MP, S], dt.bfloat16)
                kT2 = const.tile([128, MP, S], dt.bfloat16)
                v_sb = const.tile([128, TC, HPC, Dh + 1], dt.bfloat16)

                # ---- input DMAs ----
                nc.sync.dma_start(xT[:, :, :], xT_d[:].rearrange("(a p) c -> p a c", p=128))
                nc.sync.dma_start(wq[:, :, :], wq_d[:].rearrange("(a p) c -> p a c", p=128))
                nc.sync.dma_start(wk[:, :, :], wk_d[:].rearrange("(a p) c -> p a c", p=128))
                nc.sync.dma_start(wv[:, :, :], wv_d[:].rearrange("(a p) c -> p a c", p=128))
                nc.sync.dma_start(wo[:, :, :], wo_d[:])
                nc.sync.dma_start(bqk[:, 0:MP], bq_d[:])
                nc.sync.dma_start(bqk[:, MP:2 * MP], bk_d[:])
                nc.sync.dma_start(onesbv[0:1, 0:HPC * Dh], bv_d[:])
                nc.vector.memset(onesbv[0:1, HPC * Dh:], 1.0)
                nc.vector.memset(v_sb[:, :, :, :], 1.0)
                if dummy_d is not None:
                    nc.sync.dma_start(dummy_d[:, :], onesbv[0:1, 0:4])

                # ---- projections: kT2 (layout [d(2 heads), s]) + v, all s ----
                def proj_qk(w_sb, boff, dst, m, sc):
                    ssl = slice(sc * SCW, (sc + 1) * SCW)
                    p = psum_z.tile([128, SCW], dt.float32, tag="z")
                    for k in range(KE):
                        nc.tensor.matmul(
                            p[:, :],
                            w_sb[:, k, m * 128:(m + 1) * 128],
                            xT[:, k, ssl],
                            start=(k == 0), stop=(k == KE - 1),
                        )
                    nc.vector.tensor_scalar_add(
                        dst[:, m, ssl], p[:, :], bqk[:, boff + m:boff + m + 1]
                    )

                for m in range(MP):
                    for sc in range(SC):
                        proj_qk(wk, MP, kT2, m, sc)

                # v projection (natural layout [t, hd] + bias + ones col)
                for t in range(TC):
                    tsl = slice(t * TCW, (t + 1) * TCW)
                    p = psum_z.tile([128, HPC * Dh], dt.float32, tag="z")
                    for k in range(KE):
                        nc.tensor.matmul(
                            p[:, :], xT[:, k, tsl], wv[:, k, :],
                            start=(k == 0), stop=False,
                        )
                    nc.tensor.matmul(
                        p[:, :], onesbv[0:1, HPC * Dh:HPC * Dh + 128],
                        onesbv[0:1, 0:HPC * Dh], start=False, stop=True,
                    )
                    nc.vector.tensor_copy(
                        v_sb[:, t, :, 0:Dh],
                        p[:, :].rearrange("p (h c) -> p h c", c=Dh),
                    )

                # ---- attention + output projection, per s-chunk ----
                def emit_norm(h, pz, zT):
                    cs = norm1.tile([1, SCW], dt.bfloat16, tag="cs")
                    nc.vector.tensor_copy(cs[0:1, :], pz[Dh:Dh + 1, :])
                    pbc = psum_z.tile([Dh, SCW], dt.float32, tag="z")
                    nc.tensor.matmul(
                        pbc[:, :], onesbv[0:1, HPC * Dh:HPC * Dh + Dh], cs[0:1, :],
                        start=True, stop=True,
                    )
                    bch = norm1.tile([Dh, SCW], dt.float32, tag="bch")
                    nc.vector.reciprocal(bch[:, :], pbc[:, :])
                    nc.vector.tensor_tensor(
                        zT[:, h, :], pz[0:Dh, :], bch[:, :], OP.mult
                    )

                def emit_pair(m, sc, prev, zT):
                    """Scores+exp for head-pair m (rows 0-63 / 64-127 run
                    concurrently), interleaved with AV of the previous pair.
                    prev = (m_prev, eT2_prev) or None. Returns (m, eT2)."""
                    ssl = slice(sc * SCW, (sc + 1) * SCW)
                    pz_e = pz_o = None
                    if prev is not None:
                        pz_e = psum_z.tile([Dh + 1, SCW], dt.float32, tag="z")
                        pz_o = psum_z.tile([Dh + 1, SCW], dt.float32, tag="z")
                    if m is not None:
                        eT2 = work.tile([128, TC, 2, SCW], dt.bfloat16, tag="expT")
                    else:
                        eT2 = None
                    for t in range(TC):
                        if eT2 is not None:
                            pst = psum_sT.tile([128, 2 * SCW], dt.float32, tag="sT")
                            for j in range(2):
                                hoff = j * Dh
                                nc.tensor.matmul(
                                    pst[:, j * SCW:(j + 1) * SCW],
                                    kT2[hoff:hoff + Dh, m, t * TCW:(t + 1) * TCW],
                                    qT2[hoff:hoff + Dh, m, ssl],
                                    start=True, stop=True,
                                    tile_position=(hoff, 0),
                                )
                            nc.scalar.activation(
                                eT2[:, t, :, :], pst[:, :], AF.Exp, scale=0.125,
                            )
                        if prev is not None:
                            mp_, eT2p = prev
                            nc.tensor.matmul(
                                pz_e[:, :], v_sb[:, t, 2 * mp_, :],
                                eT2p[:, t, 0, :],
                                start=(t == 0), stop=(t == TC - 1),
                            )
                            nc.tensor.matmul(
                                pz_o[:, :], v_sb[:, t, 2 * mp_ + 1, :],
                                eT2p[:, t, 1, :],
                                start=(t == 0), stop=(t == TC - 1),
                            )
                    if prev is not None:
                        emit_norm(2 * prev[0], pz_e, zT)
                        emit_norm(2 * prev[0] + 1, pz_o, zT)
                    return (m, eT2)

                for sc in range(SC):
                    ssl = slice(sc * SCW, (sc + 1) * SCW)
                    for m in range(MP):
                        proj_qk(wq, 0, qT2, m, sc)
                    if parts < 2:
                        continue
                    zT = zpool.tile([Dh, HPC, SCW], dt.bfloat16, tag="zT")
                    prev = None
                    for m in range(MP):
                        prev = emit_pair(m, sc, prev, zT)
                    emit_pair(None, sc, prev, zT)
                    if parts < 3:
                        continue
                    for ec in range(ECN):
                        po = psum_z.tile([128, SCW], dt.float32, tag="z")
                        for h in range(HPC):
                            nc.tensor.matmul(
                                po[:, :],
                                wo[:, h, ec * 128:(ec + 1) * 128],
                                zT[:, h, :],
                                start=(h == 0), stop=(h == HPC - 1),
                            )
                        ob = work.tile([128, SCW], dt.float32, tag="ob")
                        nc.vector.tensor_copy(ob[:, :], po[:, :])
                        nc.sync.dma_start(
                            oT_d[ec * 128:(ec + 1) * 128, ssl], ob[:, :]
                        )

    nc.compile()
    names = {
        "xT": xT_d.name, "wq": wq_d.name, "wk": wk_d.name, "wv": wv_d.name,
        "wo": wo_d.name, "bq": bq_d.name, "bk": bk_d.name, "bv": bv_d.name,
        "oT": oT_d.name,
    }
    return nc, names


def get_program():
    global _PROG
    if _PROG is None:
        _PROG = _build_program()
    return _PROG


def make_in_maps(x, Wq, bq, Wk, bk, Wv, bv, Wo, names):
    """Host-side sharding: per-core input dict (bf16 casts + layout prep)."""
    in_maps = []
    for c in range(N_CORES):
        b, g = divmod(c, 2)
        hsl = slice(g * HPC, (g + 1) * HPC)
        xT_c = np.ascontiguousarray(x[b].T).astype(BF16)                 # [E, S]
        wq_c = np.ascontiguousarray(
            Wq[hsl].transpose(1, 0, 2).reshape(E, HPC * Dh)).astype(BF16)
        wk_c = np.ascontiguousarray(
            Wk[hsl].transpose(1, 0, 2).reshape(E, HPC * Dh)).astype(BF16)
        wv_c = np.ascontiguousarray(
            Wv[hsl].transpose(1, 0, 2).reshape(E, HPC * Dh)).astype(BF16)
        # Wo rows for this head group, packed [Dh, HPC, E] (head on free axis)
        wo_c = np.ascontiguousarray(
            Wo[g * HPC * Dh:(g + 1) * HPC * Dh].reshape(HPC, Dh, E)
            .transpose(1, 0, 2)).astype(BF16)
        bq_c = np.ascontiguousarray(bq[hsl].reshape(MP, 128).T).astype(np.float32)
        bk_c = np.ascontiguousarray(bk[hsl].reshape(MP, 128).T).astype(np.float32)
        bv_c = bv[hsl].reshape(1, HPC * Dh).astype(BF16)
        in_maps.append({
            names["xT"]: xT_c, names["wq"]: wq_c, names["wk"]: wk_c,
            names["wv"]: wv_c, names["wo"]: wo_c, names["bq"]: bq_c,
            names["bk"]: bk_c, names["bv"]: bv_c,
        })
    return in_maps


def combine_outputs(results, bo, names):
    """Host-side unshard: sum head-group partials, transpose, add bo."""
    out = np.empty((B, S, E), np.float32)
    for b in range(B):
        oT = results[2 * b][names["oT"]] + results[2 * b + 1][names["oT"]]
        out[b] = oT.T + bo
    return out


def kernel(x, Wq, bq, Wk, bk, Wv, bv, Wo, bo):
    from concourse.bass_utils import run_bass_kernel_spmd

    nc, names = get_program()
    in_maps = make_in_maps(
        np.asarray(x), np.asarray(Wq), np.asarray(bq), np.asarray(Wk),
        np.asarray(bk), np.asarray(Wv), np.asarray(bv), np.asarray(Wo), names,
    )
    res = run_bass_kernel_spmd(nc, in_maps, core_ids=list(range(N_CORES)))
    return combine_outputs(res.results, np.asarray(bo, np.float32), names)


# revision 20
# speedup vs baseline: 1.0688x; 1.0688x over previous
"""Trainium2 Bass kernel: 16-head MHA (B=4, S=2048, E=1024, Dh=64), 8 cores.

Sharding: core c handles batch b = c//2 and head-group g = c%2 (8 heads).
Each core computes its 8 heads' attention plus the partial output
projection in transposed layout oT[e, s]; the host sums the two
head-group partials per batch, transposes, and adds bo.

Per-core dataflow (all matmuls bf16 with fp32 PSUM accumulation):
  qT/kT[d, s]  = Wq/Wk.T @ xT          (per head-pair, d stacked 2x64)
  v[t, hd]     = xT.T @ Wv + ones.T@bv (natural layout, + ones col for colsum)
  scoresT[t,s] = kT.T @ qT   (row-tiled: 2 heads in rows 0-63 / 64-127)
  expT         = exp(0.125 * scoresT)  (ScalarE, cast to bf16)
  zT_un[d,s],colsum[s] = v_aug.T @ expT  (M=65: row 64 = colsum)
  zT           = zT_un * bcast(1/colsum) (bcast via K=1 matmul)
  oT[e, s]    += Wo_h.T @ zT_h  (accumulated over the core's 8 heads)
"""

import numpy as np
import ml_dtypes

B, S, E = 4, 2048, 1024
H, Dh = 16, 64
N_CORES = 8
HPC = 8          # heads per core
MP = 4           # head-pairs per core
SC, SCW = 4, 512  # s-chunks
TC, TCW = 16, 128  # t-chunks
KE = 8           # k-tiles over E
ECN = 8          # e-chunks of 128 (outT partition tiles)

BF16 = ml_dtypes.bfloat16

_PROG = None


def _build_program(repeats=None, timing=False, parts=3):
    """Emit the Bass/Tile program. Returns (nc, names_dict).

    repeats: if set, wrap the whole body in a For_i loop (for marginal
    per-iteration HW timing; not used by the graded kernel() path).
    timing: demote the real output to internal DRAM and expose a tiny
    dummy output instead, so timing calls don't pay output transfers.
    parts: 1 = projections only, 2 = + attention, 3 = full (default).
    """
    from contextlib import ExitStack

    import concourse.mybir as mybir
    import concourse.tile as tile
    from concourse import bacc

    dt = mybir.dt
    AF = mybir.ActivationFunctionType
    OP = mybir.AluOpType

    nc = bacc.Bacc(None, target_bir_lowering=False, debug=False)
    with tile.TileContext(nc) as tc:
        with tc.tile_pool(name="dram", bufs=1, space="DRAM") as dram:
            xT_d = dram.tile([E, S], dt.bfloat16, kind="ExternalInput")
            wq_d = dram.tile([E, HPC * Dh], dt.bfloat16, kind="ExternalInput")
            wk_d = dram.tile([E, HPC * Dh], dt.bfloat16, kind="ExternalInput")
            wv_d = dram.tile([E, HPC * Dh], dt.bfloat16, kind="ExternalInput")
            wo_d = dram.tile([Dh, HPC, E], dt.bfloat16, kind="ExternalInput")
            bq_d = dram.tile([128, MP], dt.float32, kind="ExternalInput")
            bk_d = dram.tile([128, MP], dt.float32, kind="ExternalInput")
            bv_d = dram.tile([1, HPC * Dh], dt.bfloat16, kind="ExternalInput")
            if timing:
                oT_d = dram.tile([E, S], dt.float32, kind="Internal")
                dummy_d = dram.tile([1, 4], dt.bfloat16, kind="ExternalOutput")
            else:
                oT_d = dram.tile([E, S], dt.float32, kind="ExternalOutput")
                dummy_d = None

            with (
                tc.tile_pool(name="const", bufs=1) as const,
                tc.tile_pool(name="work", bufs=2) as work,
                tc.tile_pool(name="norm1", bufs=1) as norm1,
                tc.tile_pool(name="zpool", bufs=1) as zpool,
                tc.tile_pool(name="psum_sT", bufs=2, space="PSUM") as psum_sT,
                tc.tile_pool(name="psum_z", bufs=4, space="PSUM") as psum_z,
                ExitStack() as _es,
            ):
                if repeats is not None:
                    _es.enter_context(tc.For_i(
                        0, repeats, 1,
                        hint_engines=(
                            mybir.EngineType.PE, mybir.EngineType.Activation,
                            mybir.EngineType.DVE, mybir.EngineType.SP,
                            mybir.EngineType.Pool,
                        ),
                    ))
                # ---- persistent SBUF ----
                xT = const.tile([128, KE, S], dt.bfloat16)
                wq = const.tile([128, KE, HPC * Dh], dt.bfloat16)
                wk = const.tile([128, KE, HPC * Dh], dt.bfloat16)
                wv = const.tile([128, KE, HPC * Dh], dt.bfloat16)
                wo = const.tile([Dh, HPC, E], dt.bfloat16)
                bqk = const.tile([128, 2 * MP], dt.float32)
                onesbv = const.tile([1, HPC * Dh + 128], dt.bfloat16)
                qT2 = const.tile([128, MP, S], dt.bfloat16)
                kT2 = const.tile([128, MP, S], dt.bfloat16)
                v_sb = const.tile([128, TC, HPC, Dh + 1], dt.bfloat16)

                # ---- input DMAs ----
                nc.sync.dma_start(xT[:, :, :], xT_d[:].rearrange("(a p) c -> p a c", p=128))
                nc.sync.dma_start(wq[:, :, :], wq_d[:].rearrange("(a p) c -> p a c", p=128))
                nc.sync.dma_start(wk[:, :, :], wk_d[:].rearrange("(a p) c -> p a c", p=128))
                nc.sync.dma_start(wv[:, :, :], wv_d[:].rearrange("(a p) c -> p a c", p=128))
                nc.sync.dma_start(wo[:, :, :], wo_d[:])
                nc.sync.dma_start(bqk[:, 0:MP], bq_d[:])
                nc.sync.dma_start(bqk[:, MP:2 * MP], bk_d[:])
                nc.sync.dma_start(onesbv[0:1, 0:HPC * Dh], bv_d[:])
                nc.vector.memset(onesbv[0:1, HPC * Dh:], 1.0)
                nc.vector.memset(v_sb[:, :, :, :], 1.0)
                if dummy_d is not None:
                    nc.sync.dma_start(dummy_d[:, :], onesbv[0:1, 0:4])

                # ---- projections: kT2 (layout [d(2 heads), s]) + v, all s ----
                def proj_qk(w_sb, boff, dst, m, sc):
                    ssl = slice(sc * SCW, (sc + 1) * SCW)
                    p = psum_z.tile([128, SCW], dt.float32, tag="z")
                    for k in range(KE):
                        nc.tensor.matmul(
                            p[:, :],
                            w_sb[:, k, m * 128:(m + 1) * 128],
                            xT[:, k, ssl],
                            start=(k == 0), stop=(k == KE - 1),
                        )
                    nc.vector.tensor_scalar_add(
                        dst[:, m, ssl], p[:, :], bqk[:, boff + m:boff + m + 1]
                    )

                for m in range(MP):
                    for sc in range(SC):
                        proj_qk(wk, MP, kT2, m, sc)

                # v projection (natural layout [t, hd] + bias + ones col)
                for t in range(TC):
                    tsl = slice(t * TCW, (t + 1) * TCW)
                    p = psum_z.tile([128, HPC * Dh], dt.float32, tag="z")
                    for k in range(KE):
                        nc.tensor.matmul(
                            p[:, :], xT[:, k, tsl], wv[:, k, :],
                            start=(k == 0), stop=False,
                        )
                    nc.tensor.matmul(
                        p[:, :], onesbv[0:1, HPC * Dh:HPC * Dh + 128],
                        onesbv[0:1, 0:HPC * Dh], start=False, stop=True,
                    )
                    nc.vector.tensor_copy(
                        v_sb[:, t, :, 0:Dh],
                        p[:, :].rearrange("p (h c) -> p h c", c=Dh),
                    )

                # ---- attention + output projection, per s-chunk ----
                def emit_norm(h, pz, zT):
                    cs = norm1.tile([1, SCW], dt.bfloat16, tag="cs")
                    nc.vector.tensor_copy(cs[0:1, :], pz[Dh:Dh + 1, :])
                    pbc = psum_z.tile([Dh, SCW], dt.float32, tag="z")
                    nc.tensor.matmul(
                        pbc[:, :], onesbv[0:1, HPC * Dh:HPC * Dh + Dh], cs[0:1, :],
                        start=True, stop=True,
                    )
                    bch = norm1.tile([Dh, SCW], dt.float32, tag="bch")
                    nc.vector.reciprocal(bch[:, :], pbc[:, :])
                    nc.vector.tensor_tensor(
                        zT[:, h, :], pz[0:Dh, :], bch[:, :], OP.mult
                    )

                def emit_outproj(sc, zT):
                    ssl = slice(sc * SCW, (sc + 1) * SCW)
                    for ec in range(ECN):
                        po = psum_z.tile([128, SCW], dt.float32, tag="z")
                        for h in range(HPC):
                            nc.tensor.matmul(
                                po[:, :],
                                wo[:, h, ec * 128:(ec + 1) * 128],
                                zT[:, h, :],
                                start=(h == 0), stop=(h == HPC - 1),
                            )
                        ob = work.tile([128, SCW], dt.float32, tag="ob")
                        nc.vector.tensor_copy(ob[:, :], po[:, :])
                        nc.sync.dma_start(
                            oT_d[ec * 128:(ec + 1) * 128, ssl], ob[:, :]
                        )

                def emit_pair(cur, prev, zTs):
                    """Scores+exp for head-pair cur=(sc, m) (rows 0-63 /
                    64-127 run concurrently), interleaved with AV of the
                    previous pair. prev = (sc, m, eT2) or None."""
                    pz_e = pz_o = None
                    if prev is not None:
                        pz_e = psum_z.tile([Dh + 1, SCW], dt.float32, tag="z")
                        pz_o = psum_z.tile([Dh + 1, SCW], dt.float32, tag="z")
                    eT2 = None
                    if cur is not None:
                        sc, m = cur
                        ssl = slice(sc * SCW, (sc + 1) * SCW)
                        eT2 = work.tile([128, TC, 2, SCW], dt.bfloat16, tag="expT")
                    for t in range(TC):
                        if eT2 is not None:
                            pst = psum_sT.tile([128, 2 * SCW], dt.float32, tag="sT")
                            for j in range(2):
                                hoff = j * Dh
                                nc.tensor.matmul(
                                    pst[:, j * SCW:(j + 1) * SCW],
                                    kT2[hoff:hoff + Dh, m, t * TCW:(t + 1) * TCW],
                                    qT2[hoff:hoff + Dh, m, ssl],
                                    start=True, stop=True,
                                    tile_position=(hoff, 0),
                                )
                            nc.scalar.activation(
                                eT2[:, t, :, :], pst[:, :], AF.Exp, scale=0.125,
                            )
                        if prev is not None:
                            scp, mp_, eT2p = prev
                            nc.tensor.matmul(
                                pz_e[:, :], v_sb[:, t, 2 * mp_, :],
                                eT2p[:, t, 0, :],
                                start=(t == 0), stop=(t == TC - 1),
                            )
                            nc.tensor.matmul(
                                pz_o[:, :], v_sb[:, t, 2 * mp_ + 1, :],
                                eT2p[:, t, 1, :],
                                start=(t == 0), stop=(t == TC - 1),
                            )
                    if prev is not None:
                        scp, mp_, _ = prev
                        emit_norm(2 * mp_, pz_e, zTs[scp])
                        emit_norm(2 * mp_ + 1, pz_o, zTs[scp])
                        if mp_ == MP - 1 and parts >= 3:
                            emit_outproj(scp, zTs[scp])
                    if cur is None:
                        return None
                    return (cur[0], cur[1], eT2)

                if parts < 2:
                    for sc in range(SC):
                        for m in range(MP):
                            proj_qk(wq, 0, qT2, m, sc)
                else:
                    pairs = [(sc, m) for sc in range(SC) for m in range(MP)]
                    zTs = {}
                    prev = None
                    proj_qk(wq, 0, qT2, pairs[0][1], pairs[0][0])
                    for i, cur in enumerate(pairs):
                        sc, m = cur
                        if m == 0:
                            zTs[sc] = zpool.tile(
                                [Dh, HPC, SCW], dt.bfloat16, tag="zT",
                                name=f"zT_{sc}")
                        if i + 1 < len(pairs):
                            proj_qk(wq, 0, qT2, pairs[i + 1][1], pairs[i + 1][0])
                        prev = emit_pair(cur, prev, zTs)
                    emit_pair(None, prev, zTs)

    nc.compile()
    names = {
        "xT": xT_d.name, "wq": wq_d.name, "wk": wk_d.name, "wv": wv_d.name,
        "wo": wo_d.name, "bq": bq_d.name, "bk": bk_d.name, "bv": bv_d.name,
        "oT": oT_d.name,
    }
    return nc, names


def get_program():
    global _PROG
    if _PROG is None:
        _PROG = _build_program()
    return _PROG


def make_in_maps(x, Wq, bq, Wk, bk, Wv, bv, Wo, names):
    """Host-side sharding: per-core input dict (bf16 casts + layout prep)."""
    in_maps = []
    for c in range(N_CORES):
        b, g = divmod(c, 2)
        hsl = slice(g * HPC, (g + 1) * HPC)
        xT_c = np.ascontiguousarray(x[b].T).astype(BF16)                 # [E, S]
        wq_c = np.ascontiguousarray(
            Wq[hsl].transpose(1, 0, 2).reshape(E, HPC * Dh)).astype(BF16)
        wk_c = np.ascontiguousarray(
            Wk[hsl].transpose(1, 0, 2).reshape(E, HPC * Dh)).astype(BF16)
        wv_c = np.ascontiguousarray(
            Wv[hsl].transpose(1, 0, 2).reshape(E, HPC * Dh)).astype(BF16)
        # Wo rows for this head group, packed [Dh, HPC, E] (head on free axis)
        wo_c = np.ascontiguousarray(
            Wo[g * HPC * Dh:(g + 1) * HPC * Dh].reshape(HPC, Dh, E)
            .transpose(1, 0, 2)).astype(BF16)
        bq_c = np.ascontiguousarray(bq[hsl].reshape(MP, 128).T).astype(np.float32)
        bk_c = np.ascontiguousarray(bk[hsl].reshape(MP, 128).T).astype(np.float32)
        bv_c = bv[hsl].reshape(1, HPC * Dh).astype(BF16)
        in_maps.append({
            names["xT"]: xT_c, names["wq"]: wq_c, names["wk"]: wk_c,
            names["wv"]: wv_c, names["wo"]: wo_c, names["bq"]: bq_c,
            names["bk"]: bk_c, names["bv"]: bv_c,
        })
    return in_maps


def combine_outputs(results, bo, names):
    """Host-side unshard: sum head-group partials, transpose, add bo."""
    out = np.empty((B, S, E), np.float32)
    for b in range(B):
        oT = results[2 * b][names["oT"]] + results[2 * b + 1][names["oT"]]
        out[b] = oT.T + bo
    return out


def kernel(x, Wq, bq, Wk, bk, Wv, bv, Wo, bo):
    from concourse.bass_utils import run_bass_kernel_spmd

    nc, names = get_program()
    in_maps = make_in_maps(
        np.asarray(x), np.asarray(Wq), np.asarray(bq), np.asarray(Wk),
        np.asarray(bk), np.asarray(Wv), np.asarray(bv), np.asarray(Wo), names,
    )
    res = run_bass_kernel_spmd(nc, in_maps, core_ids=list(range(N_CORES)))
    return combine_outputs(res.results, np.asarray(bo, np.float32), names)
